# revision 5
# baseline (speedup 1.0000x reference)
# BatchGAT Trainium2 Bass kernel.
#
# Reference computation (per batch b, head hd):
#   hp = h[b] @ w[hd]                      [n, 64]
#   t = tanh(hp)
#   s = t @ a_src[hd];  d = t @ a_dst[hd]  [n]
#   attn[i,j] = softmax_j(leaky_relu(s[i] + d[j], 0.2))
#   out = attn @ hp + bias_p
#
# Key identity used here: with es=exp(s), es2=exp(0.2 s), ed=exp(d),
# ed2=exp(0.2 d):
#   exp(leaky_relu(s_i + d_j)) = max(es_i*ed_j, es2_i*ed2_j)
# (for x>=0 exp(x)>=exp(0.2x), for x<0 exp(0.2x)>exp(x)); softmax never
# needs a max-subtraction because |s+d| <= ~40 stays in fp32/bf16 range.
# So the n^2 stage is two VectorE ops per [128, n] tile (no transcendental
# over n^2 elements), and the weighted sum + denominator come from TensorE
# matmuls with a ones-column appended to hp.
#
# Sharding: head-parallel, one head per NeuronCore (8 heads, 8 cores); each
# core computes all 4 batches of its head.

import numpy as np
from contextlib import ExitStack

import concourse.bass as bass
import concourse.tile as tile
import concourse.mybir as mybir
from concourse import bacc
from concourse.masks import make_identity
from concourse.bass_utils import run_bass_kernel_spmd

F32 = mybir.dt.float32
BF16 = mybir.dt.bfloat16
AF = mybir.ActivationFunctionType
ALU = mybir.AluOpType

NB = 4      # batches
NF = 64     # f_in == f_out
NH = 8      # heads == cores


def _chunks(total, size):
    out = []
    c0 = 0
    while c0 < total:
        cs = min(size, total - c0)
        out.append((c0, cs))
        c0 += cs
    return out


def build_gat_module(n=2048, nb=NB):
    nc = bacc.Bacc("TRN2", target_bir_lowering=False)

    h_t = nc.dram_tensor("h", [nb, n, NF], F32, kind="ExternalInput")
    w_t = nc.dram_tensor("w1", [NF, NF], F32, kind="ExternalInput")
    asd_t = nc.dram_tensor("asd", [NF, 2], F32, kind="ExternalInput")
    b_t = nc.dram_tensor("biasp", [NF], F32, kind="ExternalInput")
    o_t = nc.dram_tensor("out", [nb, n, NF], F32, kind="ExternalOutput")

    NT = n // 128          # 128-row tiles
    C512 = _chunks(n, 512)
    g8 = min(8, NT)        # jb group size for hp psum staging

    with tile.TileContext(nc) as tc:
        with ExitStack() as ctx:
            consts = ctx.enter_context(tc.tile_pool(name="consts", bufs=1))
            hpool = ctx.enter_context(tc.tile_pool(name="hpool", bufs=1))
            work = ctx.enter_context(tc.tile_pool(name="work", bufs=3))
            pairbuf = ctx.enter_context(tc.tile_pool(name="pairbuf", bufs=2))
            etp = ctx.enter_context(tc.tile_pool(name="etp", bufs=3))
            outp = ctx.enter_context(tc.tile_pool(name="outp", bufs=4))
            pst = ctx.enter_context(tc.tile_pool(name="pst", bufs=3, space="PSUM"))
            pacc = ctx.enter_context(tc.tile_pool(name="pacc", bufs=1, space="PSUM"))

            # ---- constants ----
            ident = consts.tile([128, 128], F32)
            make_identity(nc, ident)
            ones_l = consts.tile([1, 128], F32)
            nc.vector.memset(ones_l, 1.0)
            w_sb = consts.tile([NF, NF], F32)
            nc.sync.dma_start(out=w_sb, in_=w_t[:, :])
            asd_sb = consts.tile([NF, 2], F32)
            nc.sync.dma_start(out=asd_sb, in_=asd_t[:, :])
            bias_bc = consts.tile([128, NF], F32)
            bap = b_t[:]
            bias_bcast_ap = bass.AP(
                tensor=bap.tensor, offset=bap.offset, ap=[[0, 128]] + list(bap.ap)
            )
            nc.gpsimd.dma_start(out=bias_bc, in_=bias_bcast_ap)

            # ---- load h and transpose: hTs[b] = h[b].T  [64, n] ----
            hTs = []
            for b in range(nb):
                hT_t = hpool.tile([NF, n], F32, name=f"hT{b}")
                hTs.append(hT_t)
            for b in range(nb):
                for jc in range(NT):
                    hload = work.tile([128, NF], F32, name="hload")
                    nc.sync.dma_start(
                        out=hload,
                        in_=h_t[b, jc * 128:(jc + 1) * 128, :],
                    )
                    pstr = pst.tile([NF, 128], F32, tag="ps", name="pstr")
                    nc.tensor.transpose(pstr, hload, ident)
                    nc.vector.tensor_copy(
                        hTs[b][:, jc * 128:(jc + 1) * 128], pstr
                    )

            # ---- per (batch, head-on-this-core) pair ----
            for b in range(nb):
                hT = hTs[b][:, :]  # [64, n] = h[b].T

                # A: hp_ext[:, jb, 0:64] = hp rows, col 64 = 1.0 (denominator)
                hp_ext = pairbuf.tile([128, NT, 66], BF16, name="hp_ext")
                nc.vector.memset(hp_ext, 1.0)
                for (j0, js) in _chunks(NT, g8):
                    psA = pst.tile([128, g8, NF], F32, tag="ps", name="psA")
                    for k in range(js):
                        jb = j0 + k
                        nc.tensor.matmul(
                            psA[:, k, :],
                            lhsT=hT[:, jb * 128:(jb + 1) * 128],
                            rhs=w_sb,
                            start=True, stop=True,
                        )
                    nc.scalar.copy(hp_ext[:, j0:j0 + js, 0:NF], psA[:, 0:js, :])

                # B: T = tanh(hp.T)   [64, n]
                T_sb = pairbuf.tile([NF, n], F32, name="T_sb")
                for (c0, cs) in C512:
                    psB = pst.tile([NF, 512], F32, tag="ps", name="psB")
                    nc.tensor.matmul(
                        psB[:, 0:cs], lhsT=w_sb, rhs=hT[:, c0:c0 + cs],
                        start=True, stop=True,
                    )
                    nc.scalar.activation(T_sb[:, c0:c0 + cs], psB[:, 0:cs], AF.Tanh)

                # C: s_row [1, n] = a_src . T
                s_row = pairbuf.tile([1, n], F32, name="s_row")
                for (c0, cs) in C512:
                    psC = pst.tile([2, 512], F32, tag="ps", name="psC")
                    nc.tensor.matmul(
                        psC[:, 0:cs], lhsT=asd_sb, rhs=T_sb[:, c0:c0 + cs],
                        start=True, stop=True,
                    )
                    nc.scalar.copy(s_row[0:1, c0:c0 + cs], psC[0:1, 0:cs])

                # D: d_col [128, NT] (column layout) + exp / exp(0.2 .)
                psD = pst.tile([128, NT, 2], F32, tag="ps", name="psD")
                for jb in range(NT):
                    nc.tensor.matmul(
                        psD[:, jb, :],
                        lhsT=T_sb[:, jb * 128:(jb + 1) * 128],
                        rhs=asd_sb,
                        start=True, stop=True,
                    )
                d_col = pairbuf.tile([128, NT], F32, name="d_col")
                nc.vector.tensor_copy(d_col, psD[:, :, 1])
                ed_col = pairbuf.tile([128, NT], F32, name="ed_col")
                ed2_col = pairbuf.tile([128, NT], F32, name="ed2_col")
                nc.scalar.activation(ed_col, d_col, AF.Exp)
                nc.scalar.activation(ed2_col, d_col, AF.Exp, scale=0.2)

                # E: es_bc / es2_bc [128, n] bf16 — exp(s) broadcast across
                # partitions via ones-column outer product on TensorE.
                es_bc = pairbuf.tile([128, n], BF16, name="es_bc")
                es2_bc = pairbuf.tile([128, n], BF16, name="es2_bc")
                for (c0, cs) in C512:
                    psE = pst.tile([128, 512], F32, tag="ps", name="psE")
                    nc.tensor.matmul(
                        psE[:, 0:cs], lhsT=ones_l, rhs=s_row[0:1, c0:c0 + cs],
                        start=True, stop=True,
                    )
                    nc.scalar.activation(es_bc[:, c0:c0 + cs], psE[:, 0:cs], AF.Exp)
                    nc.scalar.activation(
                        es2_bc[:, c0:c0 + cs], psE[:, 0:cs], AF.Exp, scale=0.2
                    )

                # F: main loop — per j-tile build Et[j, i] then matmul-accumulate
                # 16 interleaved accumulation chains share 4 psum banks, which
                # the bank-granular start/stop grouping can't express — so we
                # zero the accumulator explicitly and accumulate with
                # start=False throughout.
                acc = pacc.tile([128, NT, 128], F32, name="acc")
                nc.vector.memset(acc, 0.0)
                for jb in range(NT):
                    p2 = pairbuf.tile([128, n], BF16, name="p2")
                    nc.vector.tensor_scalar_mul(p2, es2_bc, ed2_col[:, jb:jb + 1])
                    et = etp.tile([128, n], BF16, name="et")
                    nc.vector.scalar_tensor_tensor(
                        out=et, in0=es_bc, scalar=ed_col[:, jb:jb + 1], in1=p2,
                        op0=ALU.mult, op1=ALU.max,
                    )
                    for ic in range(NT):
                        nc.tensor.matmul(
                            acc[:, ic, 0:65],
                            lhsT=et[:, ic * 128:(ic + 1) * 128],
                            rhs=hp_ext[:, jb, 0:65],
                            start=False, stop=False,
                            skip_group_check=True,
                        )

                # G: out rows = acc[:, :, 0:64] / acc[:, :, 64] + bias
                for ic in range(NT):
                    r = outp.tile([128, 1], F32, name="r")
                    nc.vector.reciprocal(r, acc[:, ic, 64:65])
                    o_sb = outp.tile([128, NF], F32, name="o_sb")
                    nc.vector.scalar_tensor_tensor(
                        out=o_sb, in0=acc[:, ic, 0:NF], scalar=r, in1=bias_bc,
                        op0=ALU.mult, op1=ALU.add,
                    )
                    nc.sync.dma_start(
                        out=o_t[b, ic * 128:(ic + 1) * 128, :], in_=o_sb
                    )

    nc.compile()
    return nc


_CACHE = {}
_last_results = None


def _get_nc(n=2048, nb=NB):
    key = (n, nb)
    if key not in _CACHE:
        _CACHE[key] = build_gat_module(n, nb)
    return _CACHE[key]


def kernel(h, adj, w, a_src, a_dst, bias_p):
    global _last_results
    h = np.ascontiguousarray(np.asarray(h, dtype=np.float32))
    w = np.asarray(w, dtype=np.float32)
    a_src = np.asarray(a_src, dtype=np.float32)
    a_dst = np.asarray(a_dst, dtype=np.float32)
    bias_p = np.ascontiguousarray(np.asarray(bias_p, dtype=np.float32))
    nb, n, _ = h.shape

    nc = _get_nc(n, nb)
    in_maps = []
    for c in range(NH):
        asd = np.ascontiguousarray(
            np.concatenate([a_src[c], a_dst[c]], axis=1).astype(np.float32)
        )
        in_maps.append({
            "h": h,
            "w1": np.ascontiguousarray(w[c]),
            "asd": asd,
            "biasp": bias_p,
        })
    res = run_bass_kernel_spmd(nc, in_maps, core_ids=list(range(NH)))
    _last_results = res
    out = np.empty((nb, NH, n, NF), np.float32)
    for c in range(NH):
        out[:, c] = res.results[c]["out"]
    return out


# revision 12
# speedup vs baseline: 18.6379x; 18.6379x over previous
# BatchGAT Trainium2 Bass kernel.
#
# Reference computation (per batch b, head hd):
#   hp = h[b] @ w[hd]                      [n, 64]
#   t = tanh(hp)
#   s = t @ a_src[hd];  d = t @ a_dst[hd]  [n]
#   attn[i,j] = softmax_j(leaky_relu(s[i] + d[j], 0.2))
#   out = attn @ hp + bias_p
#
# Key identity used here: softmax_j is invariant to a per-i scale, so
# multiply numerator and denominator by exp(-0.2 s_i):
#   exp(leaky_relu(s_i + d_j)) * exp(-0.2 s_i)
#     = max(exp(0.8 s_i) * exp(d_j), exp(0.2 d_j))
# (branch selection is consistent: 0.8 s + d >= 0.2 d iff s + d >= 0, and
# exp(leaky) continuous at 0 so ties are exact). The second operand depends
# only on j — a per-partition scalar in a [j, i] tile — so the whole n^2
# stage is ONE VectorE tensor_scalar op per [128, n] tile:
#   Et = (es8_bcast * ed_j) max ed2_j          (4x-mode bf16)
# No transcendental ever touches n^2 elements; no max-subtraction is needed
# because |s|,|d| <= ~20 keeps everything in fp32/bf16 range. The weighted
# sum + softmax denominator come from TensorE matmuls with a ones-column
# appended to hp.
#
# Sharding: head-parallel, one head per NeuronCore (8 heads, 8 cores); each
# core computes all 4 batches of its head.

import numpy as np
from contextlib import ExitStack

import concourse.bass as bass
import concourse.tile as tile
import concourse.mybir as mybir
from concourse import bacc
from concourse.masks import make_identity
from concourse.bass_utils import run_bass_kernel_spmd

F32 = mybir.dt.float32
BF16 = mybir.dt.bfloat16
AF = mybir.ActivationFunctionType
ALU = mybir.AluOpType

NB = 4      # batches
NF = 64     # f_in == f_out
NH = 8      # heads == cores


def _chunks(total, size):
    out = []
    c0 = 0
    while c0 < total:
        cs = min(size, total - c0)
        out.append((c0, cs))
        c0 += cs
    return out


def build_gat_module(n=2048, nb=NB, reps=1):
    # reps > 1 repeats the whole per-pair computation (benchmarking aid:
    # wall(reps=3) - wall(reps=1) = 2 x device-exec, RPC overhead cancels).
    nc = bacc.Bacc("TRN2", target_bir_lowering=False)

    h_t = nc.dram_tensor("h", [nb, n, NF], F32, kind="ExternalInput")
    w_t = nc.dram_tensor("w1", [NF, NF], F32, kind="ExternalInput")
    asd_t = nc.dram_tensor("asd", [NF, 2], F32, kind="ExternalInput")
    b_t = nc.dram_tensor("biasp", [NF], F32, kind="ExternalInput")
    o_t = nc.dram_tensor("out", [nb, n, NF], F32, kind="ExternalOutput")

    NT = n // 128          # 128-row tiles
    C512 = _chunks(n, 512)
    g8 = min(8, NT)        # jb group size for hp psum staging

    with tile.TileContext(nc) as tc:
        with ExitStack() as ctx:
            consts = ctx.enter_context(tc.tile_pool(name="consts", bufs=1))
            hpool = ctx.enter_context(tc.tile_pool(name="hpool", bufs=1))
            work = ctx.enter_context(tc.tile_pool(name="work", bufs=3))
            pairbuf = ctx.enter_context(tc.tile_pool(name="pairbuf", bufs=2))
            etp = ctx.enter_context(tc.tile_pool(name="etp", bufs=3))
            outp = ctx.enter_context(tc.tile_pool(name="outp", bufs=4))
            pst = ctx.enter_context(tc.tile_pool(name="pst", bufs=3, space="PSUM"))
            pacc = ctx.enter_context(tc.tile_pool(name="pacc", bufs=1, space="PSUM"))

            # ---- constants ----
            ident = consts.tile([128, 128], F32)
            make_identity(nc, ident)
            ones_l = consts.tile([1, 128], F32)
            nc.vector.memset(ones_l, 1.0)
            # w and a_src|a_dst, replicated at partition 0 and 64 so matmuls
            # can pair them with hT slices at either base partition.
            w_sb = consts.tile([128, NF], F32)
            nc.sync.dma_start(out=w_sb[0:NF, :], in_=w_t[:, :])
            nc.sync.dma_start(out=w_sb[NF:128, :], in_=w_t[:, :])
            asd_sb = consts.tile([NF, 2], F32)
            nc.sync.dma_start(out=asd_sb, in_=asd_t[:, :])
            bias_bc = consts.tile([128, NF], F32)
            bap = b_t[:]
            bias_bcast_ap = bass.AP(
                tensor=bap.tensor, offset=bap.offset, ap=[[0, 128]] + list(bap.ap)
            )
            nc.gpsimd.dma_start(out=bias_bc, in_=bias_bcast_ap)

            # ---- load h and transpose, two batches packed per tile:
            # hTT[half][0:64, :] = h[2*half].T, hTT[half][64:128, :] =
            # h[2*half+1].T ----
            nhalf = nb // 2
            hTT = []
            for half in range(nhalf):
                hTT_t = hpool.tile([128, n], F32, name=f"hTT{half}")
                hTT.append(hTT_t)
            for half in range(nhalf):
                for jc in range(NT):
                    hload = work.tile([128, 128], F32, name="hload")
                    nc.sync.dma_start(
                        out=hload[:, 0:NF],
                        in_=h_t[2 * half, jc * 128:(jc + 1) * 128, :],
                    )
                    nc.sync.dma_start(
                        out=hload[:, NF:128],
                        in_=h_t[2 * half + 1, jc * 128:(jc + 1) * 128, :],
                    )
                    pstr = pst.tile([128, 128], F32, tag="ps", name="pstr")
                    nc.tensor.transpose(pstr, hload, ident)
                    nc.vector.tensor_copy(
                        hTT[half][:, jc * 128:(jc + 1) * 128], pstr
                    )

            # ---- per (batch, head-on-this-core) pair ----
            for b in [bb % nb for bb in range(nb * reps)]:
                half, bp = b // 2, NF * (b % 2)
                hT = hTT[half][bp:bp + NF, :]    # [64, n] = h[b].T
                w_b = w_sb[bp:bp + NF, :]        # w replica at matching base

                # A: hp_ext[:, jb, 0:64] = hp rows, col 64 = 1.0 (denominator)
                hp_ext = pairbuf.tile([128, NT, 66], BF16, name="hp_ext")
                nc.vector.memset(hp_ext, 1.0)
                for (j0, js) in _chunks(NT, g8):
                    psA = pst.tile([128, g8, NF], F32, tag="ps", name="psA")
                    for k in range(js):
                        jb = j0 + k
                        nc.tensor.matmul(
                            psA[:, k, :],
                            lhsT=hT[:, jb * 128:(jb + 1) * 128],
                            rhs=w_b,
                            start=True, stop=True,
                        )
                    nc.scalar.copy(hp_ext[:, j0:j0 + js, 0:NF], psA[:, 0:js, :])

                # B: T = tanh(hp.T)   [64, n]
                T_sb = pairbuf.tile([NF, n], F32, name="T_sb")
                for (c0, cs) in C512:
                    psB = pst.tile([NF, 512], F32, tag="ps", name="psB")
                    nc.tensor.matmul(
                        psB[:, 0:cs], lhsT=w_b, rhs=hT[:, c0:c0 + cs],
                        start=True, stop=True,
                    )
                    nc.scalar.activation(T_sb[:, c0:c0 + cs], psB[:, 0:cs], AF.Tanh)

                # C: s_row [1, n] = a_src . T
                s_row = pairbuf.tile([1, n], F32, name="s_row")
                for (c0, cs) in C512:
                    psC = pst.tile([2, 512], F32, tag="ps", name="psC")
                    nc.tensor.matmul(
                        psC[:, 0:cs], lhsT=asd_sb, rhs=T_sb[:, c0:c0 + cs],
                        start=True, stop=True,
                    )
                    nc.scalar.copy(s_row[0:1, c0:c0 + cs], psC[0:1, 0:cs])

                # D: d_col [128, NT] (column layout) + exp / exp(0.2 .)
                psD = pst.tile([128, NT, 2], F32, tag="ps", name="psD")
                for jb in range(NT):
                    nc.tensor.matmul(
                        psD[:, jb, :],
                        lhsT=T_sb[:, jb * 128:(jb + 1) * 128],
                        rhs=asd_sb,
                        start=True, stop=True,
                    )
                d_col = pairbuf.tile([128, NT], F32, name="d_col")
                nc.vector.tensor_copy(d_col, psD[:, :, 1])
                ed_col = pairbuf.tile([128, NT], F32, name="ed_col")
                ed2_col = pairbuf.tile([128, NT], F32, name="ed2_col")
                nc.scalar.activation(ed_col, d_col, AF.Exp)
                nc.scalar.activation(ed2_col, d_col, AF.Exp, scale=0.2)

                # E: es8_bc [128, n] bf16 = exp(0.8 s_i) broadcast across
                # partitions via ones-column outer product on TensorE.
                es8_bc = pairbuf.tile([128, n], BF16, name="es8_bc")
                for (c0, cs) in C512:
                    psE = pst.tile([128, 512], F32, tag="ps", name="psE")
                    nc.tensor.matmul(
                        psE[:, 0:cs], lhsT=ones_l, rhs=s_row[0:1, c0:c0 + cs],
                        start=True, stop=True,
                    )
                    nc.scalar.activation(
                        es8_bc[:, c0:c0 + cs], psE[:, 0:cs], AF.Exp, scale=0.8
                    )

                # F: main loop — per j-tile, the rescaled attention weights are
                #   Et[j, i] = (es8_bc[j, i] * ed_j) max ed2_j
                # one 2-scalar tensor_scalar op per tile. Then matmul-accumulate
                # over j. 16 interleaved accumulation chains share 4 psum banks,
                # which the bank-granular start/stop grouping can't express — so
                # we zero the accumulator explicitly and accumulate with
                # start=False throughout.
                acc = pacc.tile([128, NT, 128], F32, name="acc")
                nc.vector.memset(acc, 0.0)
                for jb in range(NT):
                    et = etp.tile([128, n], BF16, name="et")
                    nc.vector.tensor_scalar(
                        out=et, in0=es8_bc,
                        scalar1=ed_col[:, jb:jb + 1],
                        scalar2=ed2_col[:, jb:jb + 1],
                        op0=ALU.mult, op1=ALU.max,
                    )
                    for ic in range(NT):
                        nc.tensor.matmul(
                            acc[:, ic, 0:65],
                            lhsT=et[:, ic * 128:(ic + 1) * 128],
                            rhs=hp_ext[:, jb, 0:65],
                            start=False, stop=False,
                            skip_group_check=True,
                        )

                # G: out rows = acc[:, :, 0:64] / acc[:, :, 64] + bias
                for ic in range(NT):
                    r = outp.tile([128, 1], F32, name="r")
                    nc.vector.reciprocal(r, acc[:, ic, 64:65])
                    o_sb = outp.tile([128, NF], F32, name="o_sb")
                    nc.vector.scalar_tensor_tensor(
                        out=o_sb, in0=acc[:, ic, 0:NF], scalar=r, in1=bias_bc,
                        op0=ALU.mult, op1=ALU.add,
                    )
                    nc.sync.dma_start(
                        out=o_t[b, ic * 128:(ic + 1) * 128, :], in_=o_sb
                    )

    nc.compile()
    return nc


_CACHE = {}
_last_results = None


def _get_nc(n=2048, nb=NB):
    key = (n, nb)
    if key not in _CACHE:
        _CACHE[key] = build_gat_module(n, nb)
    return _CACHE[key]


def kernel(h, adj, w, a_src, a_dst, bias_p):
    global _last_results
    h = np.ascontiguousarray(np.asarray(h, dtype=np.float32))
    w = np.asarray(w, dtype=np.float32)
    a_src = np.asarray(a_src, dtype=np.float32)
    a_dst = np.asarray(a_dst, dtype=np.float32)
    bias_p = np.ascontiguousarray(np.asarray(bias_p, dtype=np.float32))
    nb, n, _ = h.shape

    nc = _get_nc(n, nb)
    in_maps = []
    for c in range(NH):
        asd = np.ascontiguousarray(
            np.concatenate([a_src[c], a_dst[c]], axis=1).astype(np.float32)
        )
        in_maps.append({
            "h": h,
            "w1": np.ascontiguousarray(w[c]),
            "asd": asd,
            "biasp": bias_p,
        })
    res = run_bass_kernel_spmd(nc, in_maps, core_ids=list(range(NH)))
    _last_results = res
    out = np.empty((nb, NH, n, NF), np.float32)
    for c in range(NH):
        out[:, c] = res.results[c]["out"]
    return out
